# revision 53
# baseline (speedup 1.0000x reference)
"""BandhaAttention Trainium2 kernel.

Sharding: 8 cores = 2 (batch) x 4 (head groups of 4 heads).
Per core: qkv projection in compensated fp8 (hi+lo splits of x and 64x-scaled
weights, DoubleRow pair-matmuls: main chain pairs hi*hi over kc pairs, cross
chain pairs lo*hi + hi*lo per kc) with the pair-0 q/k groups contraction-
streamed against the chunked input DMAs; gated q, causal attention via
transposed scores (k on partitions) in bf16, exp on ACT, AV with V-stationary
matmuls (ones column -> softmax sums for free), normalization via gpsimd
partition_broadcast, out-projection row-sharded, bf16 partial outputs summed
on host.
"""

import os
import sys

import numpy as np

for p in ("/opt/trn_rl_repo", "/opt/trn_rl_repo/concourse"):
    if p not in sys.path and os.path.isdir(p):
        sys.path.insert(0, p)

import ml_dtypes

import concourse.bacc as bacc
import concourse.mybir as mybir
from concourse.bass_utils import run_bass_kernel_spmd
from concourse.tile import TileContext

BF16 = mybir.dt.bfloat16
FP8 = mybir.dt.float8e4
F32 = mybir.dt.float32
AF = mybir.ActivationFunctionType
DR = mybir.MatmulPerfMode.DoubleRow

T = 2048
D = 1024
HD = 64
NH_LOC = 4      # heads per core
DL = NH_LOC * HD  # 256 local qkv channels
KT = D // 128   # 8 contraction chunks
NQ = T // 512   # 4 tq chunks of 512
NTT = T // 128  # 16 tiles of 128
WS = 64.0       # fp8 weight prescale

TALA = [5, 6, 7, 8]

LAST = None  # last BassKernelResults (for profiling from test.py)


def build_nc(reps=1):
    nc = bacc.Bacc("TRN2", target_bir_lowering=False)
    # fp8 blocks: xt (kc, [hi, lo], T); weights (kc, [lo, hi], cols), x64
    xt8_d = nc.dram_tensor("xt8", [128, KT * 2 * T], FP8, kind="ExternalInput")
    wqk8_d = nc.dram_tensor("wqk8", [128, KT * 2 * 2 * DL], FP8,
                            kind="ExternalInput")
    wv8_d = nc.dram_tensor("wv8", [128, KT * 2 * DL], FP8,
                           kind="ExternalInput")
    gate_d = nc.dram_tensor("gate", [128, 2 * T], BF16, kind="ExternalInput")
    tri_d = nc.dram_tensor("tri", [128, 128], BF16, kind="ExternalInput")
    wout_d = nc.dram_tensor("wout", [DL, D], BF16, kind="ExternalInput")
    out_d = nc.dram_tensor("out", [T, D], BF16, kind="ExternalOutput")

    with TileContext(nc) as tc:
      for rep in range(reps):
        with (
            tc.tile_pool(name=f"pers{rep}", bufs=2) as pers,
            tc.tile_pool(name=f"pc1{rep}", bufs=1) as pc1,
            tc.tile_pool(name=f"pv{rep}", bufs=NTT) as pv,
            tc.tile_pool(name=f"pin{rep}", bufs=1) as pin,
        ):
            # ---- persistent SBUF tiles ----
            tri = pc1.tile([128, 128], BF16, tag="tri", name="tri")
            gate_big = pc1.tile([128, 2 * T], BF16, tag="gate",
                                name="gate_big")
            xt8 = pin.tile([128, KT * 2 * T], FP8, tag="xt8", name="xt8")
            wqk8 = pin.tile([128, KT * 2 * 2 * DL], FP8, tag="wqk8",
                            name="wqk8")
            wv8 = pin.tile([128, KT * 2 * DL], FP8, tag="wv8", name="wv8")
            wout_big = pers.tile([128, 2 * D], BF16, tag="wout",
                                 name="wout_big", bufs=1)
            qp_sb = [pers.tile([128, T], BF16, tag="qp", name="qp_sb")
                     for _ in range(2)]
            kp_sb = [pers.tile([128, T], BF16, tag="kp", name="kp_sb")
                     for _ in range(2)]
            v_all = [pv.tile([128, NH_LOC * (HD + 1)], BF16, tag="vall",
                             name="v_all") for _ in range(NTT)]
            aoT = [pers.tile([128, T], BF16, tag="aoT", name="aoT")
                   for _ in range(2)]

            vx = xt8.rearrange("p (b t) -> p b t", t=T)
            vw = wqk8.rearrange("p (b c) -> p b c", c=2 * DL)
            vv = wv8.rearrange("p (b c) -> p b c", c=DL)
            gate_sb = [gate_big[:, c * T:(c + 1) * T] for c in range(2)]

            # ---- DMA issue order: streamed kc chunks, gate mid-stream,
            # late-needed tensors last
            def dma_wqk(kc):
                nc.sync.dma_start(
                    wqk8[:, kc * 2 * 2 * DL:(kc + 1) * 2 * 2 * DL],
                    wqk8_d[:, kc * 2 * 2 * DL:(kc + 1) * 2 * 2 * DL])

            def dma_xt(kc):
                nc.sync.dma_start(
                    xt8[:, kc * 2 * T:(kc + 1) * 2 * T],
                    xt8_d[:, kc * 2 * T:(kc + 1) * 2 * T])

            dma_wqk(0)
            dma_wqk(1)
            dma_xt(0)
            dma_xt(1)
            nc.sync.dma_start(wqk8[:, 2 * 2 * 2 * DL:],
                              wqk8_d[:, 2 * 2 * 2 * DL:])
            for kc in range(2, KT):
                dma_xt(kc)
                if kc == 2:
                    nc.sync.dma_start(tri, tri_d[:, :])
            nc.sync.dma_start(gate_big[:, 0:T], gate_d[:, 0:T])
            nc.sync.dma_start(wv8, wv8_d[:, :])
            nc.sync.dma_start(gate_big[:, T:2 * T], gate_d[:, T:2 * T])
            nc.sync.dma_start(
                wout_big.rearrange("p (a c) -> p a c", c=D),
                wout_d[:, :].rearrange("(a p) c -> p a c", p=128))
            wout_sb = [wout_big[:, c * D:(c + 1) * D] for c in range(2)]

            def qk_mms(ps, m, n, first_start=True):
                ns = slice(n * 512, (n + 1) * 512)
                mslice = slice(m * 128, (m + 1) * 128)
                for a in range(4):
                    nc.tensor.matmul(
                        ps, lhsT=vw[:, 4 * a:4 * a + 2, mslice],
                        rhs=vx[:, 4 * a:4 * a + 2, ns],
                        start=(a == 0 and first_start), stop=False,
                        perf_mode=DR)
                    nc.tensor.matmul(
                        ps, lhsT=vw[:, 4 * a + 2:4 * a + 4, mslice],
                        rhs=vx[:, 4 * a + 2:4 * a + 4, ns],
                        start=False, stop=False, perf_mode=DR)
                    nc.tensor.matmul(
                        ps, lhsT=vw[:, 4 * a + 1:4 * a + 4:2, mslice],
                        rhs=vx[:, 4 * a:4 * a + 3:2, ns],
                        start=False, stop=(a == 3), perf_mode=DR)

            def qk_evac(ps, m, n):
                p, ns = m % 2, slice(n * 512, (n + 1) * 512)
                if m < 2:  # q: gate has 1/WS baked in (DVE)
                    nc.vector.tensor_mul(qp_sb[p][:, ns], ps,
                                         gate_sb[p][:, ns])
                else:      # k on ACT (idle pre-attention; DVE handles q);
                    # the 1/WS scale is folded into the exp scale
                    nc.scalar.copy(kp_sb[p][:, ns], ps)

            # ---- pass A: pair-0 q/k, contraction-streamed on 8 banks ----
            with tc.tile_pool(name=f"psA{rep}", bufs=8, space="PSUM") as psA:
                psa = {(m, n): psA.tile([128, 512], F32, tag="a", name="psa")
                       for m in (0, 2) for n in range(4)}
                for kc in range(KT):
                    for m in (0, 2):
                        for n in range(4):
                            ns = slice(n * 512, (n + 1) * 512)
                            mslice = slice(m * 128, (m + 1) * 128)
                            nc.tensor.matmul(
                                psa[(m, n)],
                                lhsT=vw[:, 2 * kc:2 * kc + 2, mslice],
                                rhs=vx[:, 2 * kc:2 * kc + 2, ns],
                                start=(kc == 0), stop=False, perf_mode=DR)
                    if kc % 2 == 1:
                        a = kc // 2
                        for n in range(4):
                            for m in (0, 2):
                                ns = slice(n * 512, (n + 1) * 512)
                                mslice = slice(m * 128, (m + 1) * 128)
                                nc.tensor.matmul(
                                    psa[(m, n)],
                                    lhsT=vw[:, 4 * a + 1:4 * a + 4:2, mslice],
                                    rhs=vx[:, 4 * a:4 * a + 3:2, ns],
                                    start=False, stop=(a == 3), perf_mode=DR)
                            if a == 3:  # evac as soon as this n completes
                                qk_evac(psa[(2, n)], 2, n)
                                qk_evac(psa[(0, n)], 0, n)

            # ---- attention phase ----
            with (
                tc.tile_pool(name=f"pexp{rep}", bufs=1) as pexp,
                tc.tile_pool(name=f"psm{rep}", bufs=2) as psm,
                tc.tile_pool(name=f"pstg{rep}", bufs=4) as pstg,
                tc.tile_pool(name=f"psq{rep}", bufs=2, space="PSUM") as psq,
                tc.tile_pool(name=f"pst{rep}", bufs=2, space="PSUM") as pst,
                tc.tile_pool(name=f"pav{rep}", bufs=2, space="PSUM") as pav,
            ):
                def do_qk8(m, n):  # pair-1 q/k group (m in 1,3)
                    ps = psq.tile([128, 512], F32, tag="psq", name="ps_qk")
                    qk_mms(ps, m, n)
                    qk_evac(ps, m, n)

                def do_v(t):  # v natural t-tile (128, 256) -> v_all
                    ps = psq.tile([128, DL], F32, tag="psq", name="ps_v")
                    ts = slice(t * 128, (t + 1) * 128)
                    for a in range(4):
                        nc.tensor.matmul(
                            ps, lhsT=vx[:, 4 * a:4 * a + 3:2, ts],
                            rhs=vv[:, 4 * a + 1:4 * a + 4:2, :],
                            start=(a == 0), stop=False, perf_mode=DR)
                        nc.tensor.matmul(
                            ps, lhsT=vx[:, 4 * a:4 * a + 2, ts],
                            rhs=vv[:, 4 * a:4 * a + 2, :],
                            start=False, stop=False, perf_mode=DR)
                        nc.tensor.matmul(
                            ps, lhsT=vx[:, 4 * a + 2:4 * a + 4, ts],
                            rhs=vv[:, 4 * a + 2:4 * a + 4, :],
                            start=False, stop=(a == 3), perf_mode=DR)
                    src = ps.rearrange("p (h c) -> p h c", c=HD)
                    dst = v_all[t].rearrange("p (h c) -> p h c", c=HD + 1)
                    nc.vector.tensor_copy(dst[:, :, 0:HD], src)
                    nc.vector.memset(dst[:, :, HD:HD + 1], 1.0)

                # (p, i) -> [128, 2*w_i] tile: head hh occupies cols
                # [hh*w_i, (hh+1)*w_i), covering tq cols [128*i, T)
                expt = {}

                def do_st_piece(p, i, c0):
                    w_i = T - 128 * i
                    if c0 == 0:
                        expt[(p, i)] = pexp.tile([128, 2 * w_i], BF16,
                                                 tag=f"e{i}", name="e_t")
                    e = expt[(p, i)]
                    w = min(512, w_i - c0)
                    st = pst.tile([128, 1024], F32, tag="st", name="st_ps")
                    for hh in range(2):
                        lo, hi = hh * 64, hh * 64 + 64
                        a = 128 * i + c0
                        nc.tensor.matmul(
                            st[:, hh * 512:hh * 512 + w],
                            lhsT=kp_sb[p][lo:hi, i * 128:(i + 1) * 128],
                            rhs=qp_sb[p][lo:hi, a:a + w],
                            start=True, stop=True,
                        )
                    # one ACT for both heads via strided output
                    eview = e.rearrange("q (hh c) -> q hh c", hh=2)
                    stview = st.rearrange("q (hh c) -> q hh c", hh=2)
                    nc.scalar.activation(
                        eview[:, :, c0:c0 + w], stview[:, :, 0:w],
                        AF.Exp, scale=0.125 / WS)
                    if c0 == 0:  # causal band mask on leading 128 cols
                        nc.gpsimd.tensor_mul(e[:, 0:128], e[:, 0:128], tri)
                        nc.gpsimd.tensor_mul(e[:, w_i:w_i + 128],
                                             e[:, w_i:w_i + 128], tri)

                av_tiles = {}

                def do_av_part(p, hh, j, i0, i1):
                    last_i = 4 * j + 3
                    if i0 == 0:
                        # pair-1 final chunk: use the idle qkv psum banks so
                        # its early matmuls can run as in-loop filler without
                        # competing with the projection accumulators
                        pool, tg = (psq, "psq") if j == 3 else (pav, "av")
                        av_tiles[(p, hh, j)] = pool.tile([128, 512], F32,
                                                         tag=tg, name="av_ps")
                    av = av_tiles[(p, hh, j)]
                    for i in range(i0, i1):
                        off = 512 * j - 128 * i
                        r = max(0, -off)  # 128*(i%4) on diagonal tiles
                        w_i = T - 128 * i
                        nc.tensor.matmul(
                            av[0:HD + 1, r:512],
                            lhsT=v_all[i][:, hh * 65 + p * 130:
                                          hh * 65 + p * 130 + 65],
                            rhs=expt[(p, i)][:, hh * w_i + off + r:
                                             hh * w_i + off + 512],
                            start=(i == 0), stop=(i == last_i),
                        )
                    if i1 != last_i + 1:
                        return
                    rc = psm.tile([1, 512], F32, tag="rc", name="rc_sb")
                    nc.vector.reciprocal(rc, av[HD:HD + 1, :])
                    bc = psm.tile([64, 512], F32, tag="bc", name="bc_sb")
                    nc.gpsimd.partition_broadcast(bc, rc)
                    nc.vector.scalar_tensor_tensor(
                        aoT[p][hh * 64:hh * 64 + 64, j * 512:(j + 1) * 512],
                        av[0:HD, :], 1.0 / WS, bc,
                        mybir.AluOpType.mult, mybir.AluOpType.mult)

                stg_tiles = {}

                def do_proj(t, n):
                    pool, tg = (psq, "psq") if t < 8 else (pav, "av")
                    po = pool.tile([128, 512], F32, tag=tg, name="po_ps")
                    for c in range(2):
                        nc.tensor.matmul(
                            po,
                            lhsT=aoT[c][:, t * 128:(t + 1) * 128],
                            rhs=wout_sb[c][:, n * 512:(n + 1) * 512],
                            start=(c == 0), stop=(c == 1),
                        )
                    if n == 0:
                        stg_tiles[t] = pstg.tile([128, D], BF16, tag="stg",
                                                 name="stg_sb")
                    stg = stg_tiles[t]
                    if t >= 12:  # ACT is idle once the exp stream ends
                        nc.scalar.copy(stg[:, n * 512:(n + 1) * 512], po)
                    else:
                        nc.vector.tensor_copy(stg[:, n * 512:(n + 1) * 512],
                                              po)
                    if t == NTT - 1:  # stream the last tile per half
                        nc.sync.dma_start(
                            out_d[t * 128:(t + 1) * 128,
                                  n * 512:(n + 1) * 512],
                            stg[:, n * 512:(n + 1) * 512])
                    elif n == 1:
                        nc.sync.dma_start(out_d[t * 128:(t + 1) * 128, :], stg)

                def qk8_unit(m, n):
                    return lambda: do_qk8(m, n)

                def v_unit(t):
                    return lambda: do_v(t)

                def st_units(p, j):
                    units = []
                    for i in range(4 * j, 4 * j + 4):
                        w_i = T - 128 * i
                        for c0 in range(0, w_i, 512):
                            units.append(
                                (lambda p=p, i=i, c0=c0: do_st_piece(p, i, c0)))
                    return units

                def av_units(p, j, split=False):
                    """Returns (early_units, last_units); last_units finish
                    the accumulation + normalization. split=True makes the
                    final parts 2 tiles for a shorter dependency tail."""
                    early, last = [], []
                    last_i = 4 * j + 3
                    bounds = list(range(0, last_i + 1, 4)) + [last_i + 1]
                    if split and j == 3:
                        bounds = [0, 4, 8, 12, 14, 16]
                    for hh in range(2):
                        for i0, i1 in zip(bounds, bounds[1:]):
                            u = (lambda p=p, hh=hh, j=j, i0=i0, i1=i1:
                                 do_av_part(p, hh, j, i0, i1))
                            (last if i1 == last_i + 1 else early).append(u)
                    return early, last

                def proj_units(j):
                    return [(lambda t=t, n=n: do_proj(t, n))
                            for t in range(4 * j, 4 * j + 4) for n in range(2)]

                def interleave(primary, filler, dense=0):
                    """dense=k: only k filler units run between primaries;
                    the rest run after the last primary (shortens the
                    exp-completion critical path on small phases)."""
                    fi = 0
                    spread = filler[:dense] if dense else filler
                    for k, pu in enumerate(primary):
                        pu()
                        target = ((k + 1) * len(spread)) // len(primary)
                        while fi < target:
                            spread[fi]()
                            fi += 1
                    while fi < len(spread):
                        spread[fi]()
                        fi += 1
                    if dense:
                        for u in filler[dense:]:
                            u()

                qk8_units = [qk8_unit(m, n) for m in (1, 3) for n in range(4)]
                v_units = [v_unit(t) for t in range(NTT)]

                def av_part(p, hh, j, i0, i1):
                    return lambda: do_av_part(p, hh, j, i0, i1)

                # pair-0 attention; pair-1 qk / v / pair-0 av as PE filler
                for j in range(3):
                    filler = []
                    if j == 0:
                        filler += qk8_units[0:3] + v_units[0:6]
                    elif j == 1:
                        e, l = av_units(0, 0)
                        filler += e + l
                        filler += qk8_units[3:6] + v_units[6:11]
                    else:
                        e, l = av_units(0, 1)
                        filler += e + l
                        filler += qk8_units[6:8] + v_units[11:16]
                    interleave(st_units(0, j), filler)
                # pair-0 j=3: run the low-tile readers of pair-0 exp tiles
                # first (they gate pair-1's exp-tile reuse); defer high-tile
                # readers into st(1,0)'s otherwise-empty filler slot
                filler = [av_part(0, hh, 3, i0, i0 + 4)
                          for hh in range(2) for i0 in (0, 4)]
                filler += [av_part(0, hh, 2, i0, i0 + 4)
                           for hh in range(2) for i0 in (0, 4)]
                interleave(st_units(0, 3), filler)
                deferred = [av_part(0, hh, 2, 8, 12) for hh in range(2)]
                deferred += [av_part(0, hh, 3, 8, 12) for hh in range(2)]
                deferred += [av_part(0, hh, 3, 12, 16) for hh in range(2)]
                # pair-1: each chunk's AV retires at the end of its own st
                # phase (exps just landed), proj(j) rides the next phase, and
                # the chunk-3 accumulation drains at end-(1,2)/(1,3) so the
                # post-exp tail is just av3-last + proj(3)
                av3_early, av3_last = av_units(1, 3, split=True)
                for j in range(NQ):
                    filler = []
                    if j == 0:
                        filler += deferred
                    else:
                        e, l = av_units(1, j - 1)
                        filler += e + l
                        filler += proj_units(j - 1)
                    if j == 3:
                        filler += av3_early
                    interleave(st_units(1, j), filler,
                               dense=3 if j == 3 else 0)
                for u in av3_last:
                    u()
                for u in proj_units(3):
                    u()
    nc.compile()
    return nc


def _hilo(a):
    fp8 = ml_dtypes.float8_e4m3
    hi = a.astype(fp8)
    lo = (a - hi.astype(np.float32)).astype(fp8)
    return hi, lo


def _blocks8(mat, order):
    """(D, C) f32 -> [128, KT*2*C] fp8 with per-kc (hi,lo) or (lo,hi) blocks."""
    hi, lo = _hilo(mat)
    sel = {"hilo": (hi, lo), "lohi": (lo, hi)}[order]
    C = mat.shape[1]
    out = np.empty((128, KT, 2, C), dtype=ml_dtypes.float8_e4m3)
    for kc in range(KT):
        for s in range(2):
            out[:, kc, s, :] = sel[s][kc * 128:(kc + 1) * 128, :]
    return np.ascontiguousarray(out.reshape(128, KT * 2 * C))


def _prep_inputs(x, w_qkv, w_out, bandha_gate):
    bf = ml_dtypes.bfloat16
    t = np.arange(T)
    gate_full = np.empty((16, T), np.float64)
    for h in range(16):
        cyc = TALA[h % len(TALA)]
        gate_full[h] = 1.0 / (1.0 + np.exp(-bandha_gate[h, t % cyc].astype(np.float64)))
    gate_full /= WS  # undo the fp8 weight prescale on q
    tri = (np.arange(128)[None, :] >= np.arange(128)[:, None]).astype(bf)

    in_maps = []
    for c in range(8):
        b, g = c // 4, c % 4
        xt = np.ascontiguousarray(x[b].T)
        wqk = np.concatenate(
            [w_qkv[:, g * DL:(g + 1) * DL],
             w_qkv[:, D + g * DL:D + (g + 1) * DL]], axis=1) * WS
        wv = w_qkv[:, 2 * D + g * DL:2 * D + (g + 1) * DL] * WS
        wout = np.ascontiguousarray(w_out[g * DL:(g + 1) * DL, :]).astype(bf)
        gb = np.repeat(gate_full[4 * g:4 * g + 4].astype(np.float32), HD,
                       axis=0).astype(bf)  # (256, T) -> [128, 2, T]
        gate = np.ascontiguousarray(
            gb.reshape(2, 128, T).transpose(1, 0, 2).reshape(128, 2 * T))
        in_maps.append({
            "xt8": _blocks8(xt, "hilo"),
            "wqk8": _blocks8(np.ascontiguousarray(wqk), "lohi"),
            "wv8": _blocks8(np.ascontiguousarray(wv), "lohi"),
            "gate": gate,
            "wout": wout, "tri": tri})
    return in_maps


def kernel(**inputs):
    global LAST
    x = np.asarray(inputs["x"], np.float32)
    w_qkv = np.asarray(inputs["w_qkv"], np.float32)
    w_out = np.asarray(inputs["w_out"], np.float32)
    bandha_gate = np.asarray(inputs["bandha_gate"], np.float32)

    in_maps = _prep_inputs(x, w_qkv, w_out, bandha_gate)
    nc = build_nc()
    res = run_bass_kernel_spmd(
        nc, in_maps, core_ids=list(range(8)),
        trace=os.environ.get("BANDHA_TRACE") == "1",
    )
    LAST = res
    outs = [r["out"].astype(np.float32) for r in res.results]
    full = np.empty((2, T, D), np.float32)
    for b in range(2):
        full[b] = outs[4 * b] + outs[4 * b + 1] + outs[4 * b + 2] + outs[4 * b + 3]
    return full
